# revision 12
# baseline (speedup 1.0000x reference)
"""Multi-head attention (B=4, S=2048, D=1024, H=16) on 8 TRN2 NeuronCores.

Data-parallel over the 64 (batch, head) attention pairs: 8 pairs per core.
Per pair, on-device (all matmul inputs bf16, PSUM accumulation fp32):
  q^T = [Wq.T; bq].T @ [X_q^T; 1]           -> [64, 2048]
  k^T = [Wk.T; bk].T @ [X_k^T; 1]           -> [64, 2048]
  v'  = [X_v^T; 1].T @ [[Wv.T, 0]; [bv, 1]] -> [2048, 65]  (ones column)
  S^T[ki, qi] = k^T.T @ q^T  (contraction over head dim 64)
  P^T = exp(S^T / 8)   -- split between ScalarE (exact spline exp) and
                          VectorE (Schraudolph bf16-bit exp) so neither
                          engine gates the PE
  out'[d', qi] = v'.T @ P^T                 -> [65, 2048]
Row 64 of out' is the softmax denominator (via the ones column of v');
the host divides and reassembles. exp needs no max subtraction: scores/8
has stddev ~0.33 for these inputs, far from fp32 overflow.

Attention runs as FOUR independent qi-quarter streams (512 cols = 1 PSUM
bank each: 4 scores tiles + 4 PV accumulators = 8 banks). Per ki-chunk
the PE issues 4 scores + 4 PV matmuls; each PV depends on an exp that
had >=3 matmul slots of cover, so the in-order PE never stalls and its
HAM clock stays at the warm 2.4 GHz.
"""

import numpy as np
import ml_dtypes

B, S, D, H = 4, 2048, 1024, 16
HD = D // H  # 64
N_CORES = 8
PAIRS_PER_CORE = (B * H) // N_CORES  # 8
KC = S // 128  # 16 ki chunks of 128
NQ = 4         # qi quarters of 512
BF16 = ml_dtypes.bfloat16

# exp split within each 512-wide quarter: ScalarE does [0, E_ACT),
# VectorE does [E_ACT, 512).
E_ACT = 288
# Schraudolph constants for bf16-bit exp(s/8): bits = s*A + B -> int16
SCH_A = 16 * 1.4426950408889634  # 128*log2(e)/8
SCH_B = 16256.0 - 5.5 - 3.0      # bias centered so rel err ~ +-1.7%

_COMPILED = {}


def _build_nc():
    import concourse.bass as bass  # noqa: F401
    import concourse.mybir as mybir
    import concourse.tile as tile
    from concourse import bacc

    f32 = mybir.dt.float32
    bf16 = mybir.dt.bfloat16
    i16 = mybir.dt.int16

    nc = bacc.Bacc("TRN2", num_devices=N_CORES)
    xq = nc.declare_dram_parameter("xq", [PAIRS_PER_CORE, HD + 1, S], bf16, isOutput=False)
    xk = nc.declare_dram_parameter("xk", [PAIRS_PER_CORE, HD + 1, S], bf16, isOutput=False)
    xv = nc.declare_dram_parameter("xv", [PAIRS_PER_CORE, HD + 1, S], bf16, isOutput=False)
    wq = nc.declare_dram_parameter("wq", [HD + 1, HD], bf16, isOutput=False)
    wk = nc.declare_dram_parameter("wk", [HD + 1, HD], bf16, isOutput=False)
    wv = nc.declare_dram_parameter("wv", [HD + 1, HD + 1], bf16, isOutput=False)
    out = nc.declare_dram_parameter("out", [PAIRS_PER_CORE, HD + 1, S], f32, isOutput=True)

    EXP = mybir.ActivationFunctionType.Exp
    MULT = mybir.AluOpType.mult
    ADD = mybir.AluOpType.add

    with tile.TileContext(nc) as tc:
        with (
            tc.tile_pool(name="consts", bufs=1) as consts,
            tc.tile_pool(name="ins", bufs=2) as ins_pool,
            tc.tile_pool(name="qk", bufs=2) as qk_pool,
            tc.tile_pool(name="vp", bufs=2) as v_pool,
            tc.tile_pool(name="pt", bufs=8) as pt_pool,
            tc.tile_pool(name="ob", bufs=4) as out_pool,
            tc.tile_pool(name="sc", bufs=4, space="PSUM") as sc_pool,
            tc.tile_pool(name="pv", bufs=4, space="PSUM") as pv_pool,
        ):
            w_q = consts.tile([HD + 1, HD], bf16)
            nc.sync.dma_start(out=w_q[:], in_=wq[:])
            w_k = consts.tile([HD + 1, HD], bf16)
            nc.sync.dma_start(out=w_k[:], in_=wk[:])
            w_v = consts.tile([HD + 1, HD + 1], bf16)
            nc.sync.dma_start(out=w_v[:], in_=wv[:])

            for j in range(PAIRS_PER_CORE):
                Xq = ins_pool.tile([HD + 1, S], bf16, tag="Xq")
                nc.sync.dma_start(out=Xq[:], in_=xq[j])
                Xk = ins_pool.tile([HD + 1, S], bf16, tag="Xk")
                nc.sync.dma_start(out=Xk[:], in_=xk[j])
                Xv = ins_pool.tile([HD + 1, S], bf16, tag="Xv")
                nc.sync.dma_start(out=Xv[:], in_=xv[j])

                # q^T, k^T projections: [64, 2048] bf16
                qT = qk_pool.tile([HD, S], bf16, tag="qT")
                kT = qk_pool.tile([HD, S], bf16, tag="kT")
                proj = []
                for n4 in range(4):
                    ps_q = sc_pool.tile([128, 512], f32, tag="sc")
                    ps_k = sc_pool.tile([128, 512], f32, tag="sc")
                    col = n4 * 512
                    nc.tensor.matmul(ps_q[0:HD, :], w_q[:], Xq[:, col : col + 512],
                                     start=True, stop=True)
                    nc.tensor.matmul(ps_k[0:HD, :], w_k[:], Xk[:, col : col + 512],
                                     start=True, stop=True)
                    proj.append((col, ps_q, ps_k))
                    if n4 % 2 == 1:
                        for pcol, pq, pk in proj:
                            nc.vector.tensor_copy(qT[:, pcol : pcol + 512], pq[0:HD, :])
                            nc.vector.tensor_copy(kT[:, pcol : pcol + 512], pk[0:HD, :])
                        proj = []

                # v' projection: [2048, 65] bf16, chunk c at columns c*65
                vS = v_pool.tile([128, KC * (HD + 1)], bf16, tag="vS")
                for g in range(4):
                    ps_v = sc_pool.tile([128, 4 * (HD + 1)], f32, tag="sc")
                    for c4 in range(4):
                        c = g * 4 + c4
                        nc.tensor.matmul(
                            ps_v[:, c4 * (HD + 1) : (c4 + 1) * (HD + 1)],
                            Xv[:, c * 128 : (c + 1) * 128], w_v[:],
                            start=True, stop=True,
                        )
                    nc.vector.tensor_copy(
                        vS[:, g * 4 * (HD + 1) : (g + 1) * 4 * (HD + 1)], ps_v[:]
                    )

                # attention: 4 interleaved qi-quarter streams
                pvs = [
                    pv_pool.tile([HD + 1, 512], f32, tag="pv", name=f"pv{q}")
                    for q in range(NQ)
                ]
                for c in range(KC):
                    kslice = kT[:, c * 128 : (c + 1) * 128]
                    vslice = vS[:, c * (HD + 1) : (c + 1) * (HD + 1)]
                    scs = []
                    for q in range(NQ):
                        sc = sc_pool.tile([128, 512], f32, tag="sc")
                        nc.tensor.matmul(
                            sc[:], kslice, qT[:, q * 512 : (q + 1) * 512],
                            start=True, stop=True,
                        )
                        pT = pt_pool.tile([128, 512], bf16, tag="pT")
                        nc.scalar.activation(
                            pT[:, 0:E_ACT], sc[:, 0:E_ACT], EXP, scale=0.125
                        )
                        nc.vector.tensor_scalar(
                            pT[:, E_ACT:512].bitcast(i16),
                            sc[:, E_ACT:512],
                            SCH_A, SCH_B, MULT, ADD,
                        )
                        scs.append(pT)
                    for q in range(NQ):
                        nc.tensor.matmul(
                            pvs[q][:], vslice, scs[q][:],
                            start=(c == 0), stop=(c == KC - 1),
                        )
                for q in range(NQ):
                    ob = out_pool.tile([HD + 1, 512], f32, tag="ob")
                    nc.scalar.copy(ob[:], pvs[q][:])
                    nc.sync.dma_start(
                        out=out[j, :, q * 512 : (q + 1) * 512], in_=ob[:]
                    )
    nc.finalize()
    return nc


def _get_nc():
    if "nc" not in _COMPILED:
        _COMPILED["nc"] = _build_nc()
    return _COMPILED["nc"]


def _prep_inputs(query, key_, value, Wq, bq, Wk, bk, Wv, bv):
    """Host-side repack: per (b,h) pair, [65, 2048] bf16 transposed-augmented."""
    def to_pairs(x):
        # [B, S, D] -> [B*H, HD, S] with ones row appended -> [B*H, HD+1, S]
        x = np.asarray(x, dtype=np.float32)
        x = x.reshape(B, S, H, HD).transpose(0, 2, 3, 1).reshape(B * H, HD, S)
        ones = np.ones((B * H, 1, S), dtype=np.float32)
        return np.ascontiguousarray(
            np.concatenate([x, ones], axis=1).astype(BF16)
        )

    xq_all = to_pairs(query)
    xk_all = to_pairs(key_)
    xv_all = to_pairs(value)

    Wq = np.asarray(Wq, np.float32); bq = np.asarray(bq, np.float32)
    Wk = np.asarray(Wk, np.float32); bk = np.asarray(bk, np.float32)
    Wv = np.asarray(Wv, np.float32); bv = np.asarray(bv, np.float32)
    wq_aug = np.concatenate([Wq.T, bq[None, :]], axis=0).astype(BF16)
    wk_aug = np.concatenate([Wk.T, bk[None, :]], axis=0).astype(BF16)
    wv_aug = np.zeros((HD + 1, HD + 1), np.float32)
    wv_aug[:HD, :HD] = Wv.T
    wv_aug[HD, :HD] = bv
    wv_aug[HD, HD] = 1.0
    wv_aug = wv_aug.astype(BF16)

    in_maps = []
    for i in range(N_CORES):
        sl = slice(i * PAIRS_PER_CORE, (i + 1) * PAIRS_PER_CORE)
        in_maps.append({
            "xq": np.ascontiguousarray(xq_all[sl]),
            "xk": np.ascontiguousarray(xk_all[sl]),
            "xv": np.ascontiguousarray(xv_all[sl]),
            "wq": wq_aug, "wk": wk_aug, "wv": wv_aug,
        })
    return in_maps


def _postprocess(outs):
    """outs: list of 8 arrays [8, 65, 2048] -> [B, S, D] float32."""
    full = np.concatenate(outs, axis=0)  # [64, 65, 2048]
    num = full[:, :HD, :]                # [64, 64, 2048]  (x_att^T unnormalized)
    den = full[:, HD : HD + 1, :]        # [64, 1, 2048]
    att = num / den                      # [B*H, HD, S]
    att = att.reshape(B, H, HD, S).transpose(0, 3, 1, 2).reshape(B, S, D)
    return np.ascontiguousarray(att.astype(np.float32))


def kernel(query, key_, value, Wq, bq, Wk, bk, Wv, bv, _trace=False, _res_box=None):
    from concourse.bass_utils import run_bass_kernel_spmd

    nc = _get_nc()
    in_maps = _prep_inputs(query, key_, value, Wq, bq, Wk, bk, Wv, bv)
    res = run_bass_kernel_spmd(
        nc, in_maps, core_ids=list(range(N_CORES)), trace=_trace
    )
    if _res_box is not None:
        _res_box.append(res)
    outs = [res.results[i]["out"] for i in range(N_CORES)]
    return _postprocess(outs)


# revision 17
# speedup vs baseline: 1.1829x; 1.1829x over previous
"""Multi-head attention (B=4, S=2048, D=1024, H=16) on 8 TRN2 NeuronCores.

Data-parallel over the 64 (batch, head) attention pairs: 8 pairs per core.
Per pair, on-device (all matmul inputs bf16, PSUM accumulation fp32):
  q^T = [Wq.T; bq].T @ [X_q^T; 1]           -> [64, 2048]
  k^T = [Wk.T; bk].T @ [X_k^T; 1]           -> [64, 2048]
  v'  = [X_v^T; 1].T @ [[Wv.T, 0]; [bv, 1]] -> [2048, 65]  (ones column)
  S^T[ki, qi] = k^T.T @ q^T  (contraction over head dim 64)
  P^T = exp(S^T / 8)   -- split between ScalarE (exact spline exp) and
                          VectorE (Schraudolph bf16-bit exp) so neither
                          engine gates the PE
  out'[d', qi] = v'.T @ P^T                 -> [65, 2048]
Row 64 of out' is the softmax denominator (via the ones column of v');
the host divides and reassembles. exp needs no max subtraction: scores/8
has stddev ~0.33 for these inputs, far from fp32 overflow.

Attention runs as FOUR independent qi-quarter streams (512 cols = 1 PSUM
bank each: 4 scores tiles + 4 PV accumulators = 8 banks). Per ki-chunk
the PE issues 4 scores + 4 PV matmuls; each PV depends on an exp that
had >=3 matmul slots of cover, so the in-order PE never stalls and its
HAM clock stays at the warm 2.4 GHz.
"""

import numpy as np
import ml_dtypes

B, S, D, H = 4, 2048, 1024, 16
HD = D // H  # 64
N_CORES = 8
PAIRS_PER_CORE = (B * H) // N_CORES  # 8
KC = S // 128  # 16 ki chunks of 128
NQ = 4         # qi quarters of 512
BF16 = ml_dtypes.bfloat16

# exp split within each 1024-wide scores tile: ScalarE does [0, E_ACT),
# VectorE does [E_ACT, 1024).
E_ACT = 576
# Schraudolph constants for bf16-bit exp(s/8): bits = s*A + B -> int16
SCH_A = 16 * 1.4426950408889634  # 128*log2(e)/8
SCH_B = 16256.0 - 5.5 - 3.0      # bias centered so rel err ~ +-1.7%

_COMPILED = {}


def _build_nc():
    import concourse.bass as bass  # noqa: F401
    import concourse.mybir as mybir
    import concourse.tile as tile
    from concourse import bacc

    f32 = mybir.dt.float32
    bf16 = mybir.dt.bfloat16
    i16 = mybir.dt.int16

    nc = bacc.Bacc("TRN2", num_devices=N_CORES)
    xq = nc.declare_dram_parameter("xq", [PAIRS_PER_CORE, HD + 1, S], bf16, isOutput=False)
    xk = nc.declare_dram_parameter("xk", [PAIRS_PER_CORE, HD + 1, S], bf16, isOutput=False)
    xv = nc.declare_dram_parameter("xv", [PAIRS_PER_CORE, HD + 1, S], bf16, isOutput=False)
    wq = nc.declare_dram_parameter("wq", [HD + 1, HD], bf16, isOutput=False)
    wk = nc.declare_dram_parameter("wk", [HD + 1, HD], bf16, isOutput=False)
    wv = nc.declare_dram_parameter("wv", [HD + 1, HD + 1], bf16, isOutput=False)
    out = nc.declare_dram_parameter("out", [PAIRS_PER_CORE, HD + 1, S], f32, isOutput=True)

    EXP = mybir.ActivationFunctionType.Exp
    MULT = mybir.AluOpType.mult
    ADD = mybir.AluOpType.add

    with tile.TileContext(nc) as tc:
        with (
            tc.tile_pool(name="consts", bufs=1) as consts,
            tc.tile_pool(name="ins", bufs=2) as ins_pool,
            tc.tile_pool(name="qk", bufs=2) as qk_pool,
            tc.tile_pool(name="vp", bufs=2) as v_pool,
            tc.tile_pool(name="pt", bufs=4) as pt_pool,
            tc.tile_pool(name="ob", bufs=4) as out_pool,
            tc.tile_pool(name="sc", bufs=3, space="PSUM") as sc_pool,
            tc.tile_pool(name="pv", bufs=2, space="PSUM") as pv_pool,
        ):
            w_q = consts.tile([HD + 1, HD], bf16)
            nc.sync.dma_start(out=w_q[:], in_=wq[:])
            w_k = consts.tile([HD + 1, HD], bf16)
            nc.sync.dma_start(out=w_k[:], in_=wk[:])
            w_v = consts.tile([HD + 1, HD + 1], bf16)
            nc.sync.dma_start(out=w_v[:], in_=wv[:])

            for j in range(PAIRS_PER_CORE):
                Xq = ins_pool.tile([HD + 1, S], bf16, tag="Xq")
                nc.sync.dma_start(out=Xq[:], in_=xq[j])
                Xk = ins_pool.tile([HD + 1, S], bf16, tag="Xk")
                nc.sync.dma_start(out=Xk[:], in_=xk[j])
                Xv = ins_pool.tile([HD + 1, S], bf16, tag="Xv")
                nc.sync.dma_start(out=Xv[:], in_=xv[j])

                # q^T, k^T projections: [64, 2048] bf16
                qT = qk_pool.tile([HD, S], bf16, tag="qT")
                kT = qk_pool.tile([HD, S], bf16, tag="kT")
                proj = []
                for n4 in range(4):
                    ps_q = sc_pool.tile([128, 512], f32, tag="sc")
                    ps_k = sc_pool.tile([128, 512], f32, tag="sc")
                    col = n4 * 512
                    nc.tensor.matmul(ps_q[0:HD, :], w_q[:], Xq[:, col : col + 512],
                                     start=True, stop=True)
                    nc.tensor.matmul(ps_k[0:HD, :], w_k[:], Xk[:, col : col + 512],
                                     start=True, stop=True)
                    proj.append((col, ps_q, ps_k))
                    if n4 % 2 == 1:
                        for pcol, pq, pk in proj:
                            nc.vector.tensor_copy(qT[:, pcol : pcol + 512], pq[0:HD, :])
                            nc.vector.tensor_copy(kT[:, pcol : pcol + 512], pk[0:HD, :])
                        proj = []

                # v' projection: [2048, 65] bf16, chunk c at columns c*65
                vS = v_pool.tile([128, KC * (HD + 1)], bf16, tag="vS")
                for g in range(4):
                    ps_v = sc_pool.tile([128, 4 * (HD + 1)], f32, tag="sc")
                    for c4 in range(4):
                        c = g * 4 + c4
                        nc.tensor.matmul(
                            ps_v[:, c4 * (HD + 1) : (c4 + 1) * (HD + 1)],
                            Xv[:, c * 128 : (c + 1) * 128], w_v[:],
                            start=True, stop=True,
                        )
                    nc.vector.tensor_copy(
                        vS[:, g * 4 * (HD + 1) : (g + 1) * 4 * (HD + 1)], ps_v[:]
                    )

                # attention: two passes over qi-halves of 1024. Per chunk:
                # one [128,1024] scores tile + split exp + 2 PV matmuls
                # into two [65,512] accumulators. PV trails scores by one
                # chunk so the in-order PE never waits on exp.
                for h2 in range(2):
                    base = h2 * 1024
                    pvs = [
                        pv_pool.tile([HD + 1, 512], f32, tag="pv", name=f"pv{q}")
                        for q in range(2)
                    ]

                    def emit_scores_exp(c):
                        kslice = kT[:, c * 128 : (c + 1) * 128]
                        sc = sc_pool.tile([128, 1024], f32, tag="sc")
                        for n in range(2):
                            col = base + n * 512
                            nc.tensor.matmul(
                                sc[:, n * 512 : (n + 1) * 512],
                                kslice, qT[:, col : col + 512],
                                start=True, stop=True,
                            )
                        pT = pt_pool.tile([128, 1024], bf16, tag="pT")
                        nc.scalar.activation(
                            pT[:, 0:E_ACT], sc[:, 0:E_ACT], EXP, scale=0.125
                        )
                        nc.vector.tensor_scalar(
                            pT[:, E_ACT:1024].bitcast(i16),
                            sc[:, E_ACT:1024],
                            SCH_A, SCH_B, MULT, ADD,
                        )
                        return pT

                    def emit_pv(c, pT):
                        vslice = vS[:, c * (HD + 1) : (c + 1) * (HD + 1)]
                        for q in range(2):
                            nc.tensor.matmul(
                                pvs[q][:], vslice,
                                pT[:, q * 512 : (q + 1) * 512],
                                start=(c == 0), stop=(c == KC - 1),
                            )

                    prev = emit_scores_exp(0)
                    for c in range(KC):
                        cur = emit_scores_exp(c + 1) if c + 1 < KC else None
                        emit_pv(c, prev)
                        prev = cur
                    for q in range(2):
                        ob = out_pool.tile([HD + 1, 512], f32, tag="ob")
                        nc.scalar.copy(ob[:], pvs[q][:])
                        nc.sync.dma_start(
                            out=out[j, :, base + q * 512 : base + (q + 1) * 512],
                            in_=ob[:],
                        )
    nc.finalize()
    return nc


def _get_nc():
    if "nc" not in _COMPILED:
        _COMPILED["nc"] = _build_nc()
    return _COMPILED["nc"]


def _prep_inputs(query, key_, value, Wq, bq, Wk, bk, Wv, bv):
    """Host-side repack: per (b,h) pair, [65, 2048] bf16 transposed-augmented."""
    def to_pairs(x):
        # [B, S, D] -> [B*H, HD, S] with ones row appended -> [B*H, HD+1, S]
        x = np.asarray(x, dtype=np.float32)
        x = x.reshape(B, S, H, HD).transpose(0, 2, 3, 1).reshape(B * H, HD, S)
        ones = np.ones((B * H, 1, S), dtype=np.float32)
        return np.ascontiguousarray(
            np.concatenate([x, ones], axis=1).astype(BF16)
        )

    xq_all = to_pairs(query)
    xk_all = to_pairs(key_)
    xv_all = to_pairs(value)

    Wq = np.asarray(Wq, np.float32); bq = np.asarray(bq, np.float32)
    Wk = np.asarray(Wk, np.float32); bk = np.asarray(bk, np.float32)
    Wv = np.asarray(Wv, np.float32); bv = np.asarray(bv, np.float32)
    wq_aug = np.concatenate([Wq.T, bq[None, :]], axis=0).astype(BF16)
    wk_aug = np.concatenate([Wk.T, bk[None, :]], axis=0).astype(BF16)
    wv_aug = np.zeros((HD + 1, HD + 1), np.float32)
    wv_aug[:HD, :HD] = Wv.T
    wv_aug[HD, :HD] = bv
    wv_aug[HD, HD] = 1.0
    wv_aug = wv_aug.astype(BF16)

    in_maps = []
    for i in range(N_CORES):
        sl = slice(i * PAIRS_PER_CORE, (i + 1) * PAIRS_PER_CORE)
        in_maps.append({
            "xq": np.ascontiguousarray(xq_all[sl]),
            "xk": np.ascontiguousarray(xk_all[sl]),
            "xv": np.ascontiguousarray(xv_all[sl]),
            "wq": wq_aug, "wk": wk_aug, "wv": wv_aug,
        })
    return in_maps


def _postprocess(outs):
    """outs: list of 8 arrays [8, 65, 2048] -> [B, S, D] float32."""
    full = np.concatenate(outs, axis=0)  # [64, 65, 2048]
    num = full[:, :HD, :]                # [64, 64, 2048]  (x_att^T unnormalized)
    den = full[:, HD : HD + 1, :]        # [64, 1, 2048]
    att = num / den                      # [B*H, HD, S]
    att = att.reshape(B, H, HD, S).transpose(0, 3, 1, 2).reshape(B, S, D)
    return np.ascontiguousarray(att.astype(np.float32))


def kernel(query, key_, value, Wq, bq, Wk, bk, Wv, bv, _trace=False, _res_box=None):
    from concourse.bass_utils import run_bass_kernel_spmd

    nc = _get_nc()
    in_maps = _prep_inputs(query, key_, value, Wq, bq, Wk, bk, Wv, bv)
    res = run_bass_kernel_spmd(
        nc, in_maps, core_ids=list(range(N_CORES)), trace=_trace
    )
    if _res_box is not None:
        _res_box.append(res)
    outs = [res.results[i]["out"] for i in range(N_CORES)]
    return _postprocess(outs)
